# revision 1
# baseline (speedup 1.0000x reference)
"""CohortAwareBlock Trainium2 kernel.

Data-parallel over batch B=8 across 8 NeuronCores (one sample per core).
Cohort routing (gather of cohort_q_w by per-sample cohort id) happens on the
host while building each core's weight tensors; the device kernel is a plain
attention block.

Numerics: fp16 matmul inputs everywhere (same PE rate as bf16, ~8x less
noise); optionally the QK projection runs as fp8-e4m3 DoubleRow matmuls
(QK_FP8) with weights pre-scaled x32 to dodge fp8 subnormals and the inverse
scale folded into the exp's `scale` argument.

Per-core structure:
  qk^T [2048, N]  (QK-gen; fp8-DR or fp16)
  v_aug [keys, h, 65] fp16 (V-gen; col 64 = 1.0 so the flipped AV emits the
                            softmax denominator per q-partition)
  per (q-quarter, head pair):
    scores -> 2-bank PSUM [128, 4x256] -> ACT exp (fp16, batched) ->
    flipped attn@v: out [128 q, 65] per (head, q-128-chunk); col 64 = den ->
    DVE reciprocal [128,1] + tensor_scalar_mul -> nm_qd fp16 [q, d] layout
  per q-quarter: DMA-XBAR transpose nm_qd -> nmT [d, q] -> proj (fp16) + bias

PE emission is software-pipelined: scores of iteration i+1 are queued before
attn@v of iteration i so the in-order PE never waits on ACT's exp; QK/V
generation and the projection fill PE slack under the ACT-bound exp window.
"""

import numpy as np

import concourse.bass as bass
import concourse.bacc as bacc
import concourse.mybir as mybir
import concourse.tile as tile
from concourse.bass_utils import run_bass_kernel_spmd

P = 128
N = 1024            # sequence length
D = 1024            # model dim
H = 16              # heads
HD = 64             # head dim
NQ = 4              # q-quarters (256 q each)
QW = N // NQ        # 256
SCALE = HD ** -0.5
NCORES = 8

QK_FP8 = True       # fp8-e4m3 DoubleRow QK-gen (err ~1.4e-2) vs fp16 (~4e-4)
WS = 32.0 if QK_FP8 else 1.0
EXP_SCALE = SCALE / (WS * WS)

F32 = mybir.dt.float32
FP16 = mybir.dt.float16
BF16 = mybir.dt.bfloat16
FP8 = mybir.dt.float8e4
DR = mybir.MatmulPerfMode.DoubleRow
EXP = mybir.ActivationFunctionType.Exp


def build_nc():
    nc = bacc.Bacc(
        "TRN2",
        target_bir_lowering=False,
        debug=False,
        num_devices=NCORES,
    )

    # ---- external I/O (per-core shards, host-prepped layouts) ----
    xt = nc.dram_tensor("xt", [P, 8, N], FP16, kind="ExternalInput")   # x^T
    if QK_FP8:
        # DoubleRow-interleaved d-dim: d = (t2*2 + j)*128 + p
        xdr = nc.dram_tensor("xdr", [P, 4, 2, N], FP8, kind="ExternalInput")
        wqk = nc.dram_tensor("wqk", [P, 16, 4, 2, P], FP8, kind="ExternalInput")
    else:
        xdr = None
        wqk = nc.dram_tensor("wqk", [P, 16, 8, P], FP16, kind="ExternalInput")
    bqk = nc.dram_tensor("bqk", [P, 16], F32, kind="ExternalInput")
    wv = nc.dram_tensor("wv", [P, 8, D], FP16, kind="ExternalInput")
    bv = nc.dram_tensor("bv", [D], BF16, kind="ExternalInput")
    wp = nc.dram_tensor("wp", [P, 8, D], FP16, kind="ExternalInput")
    bp = nc.dram_tensor("bp", [D], BF16, kind="ExternalInput")
    out = nc.dram_tensor("out", [N, D], F32, kind="ExternalOutput")

    with tile.TileContext(nc) as tc:
        kernel_body(tc, xt, xdr, wqk, bqk, wv, bv, wp, bp, out)
    nc.compile()
    return nc


def kernel_body(tc, xt, xdr, wqk, bqk, wv, bv, wp, bp, out):
    nc = tc.nc
    from contextlib import ExitStack

    with ExitStack() as ctx:
        ctx.enter_context(
            nc.allow_low_precision(reason="fp16/fp8 matmul inputs by design")
        )
        res = ctx.enter_context(tc.tile_pool(name="res", bufs=1))
        gen_ps = ctx.enter_context(tc.tile_pool(name="gen_ps", bufs=2, space="PSUM"))
        sc_ps = ctx.enter_context(tc.tile_pool(name="sc_ps", bufs=2, space="PSUM"))
        av_ps = ctx.enter_context(tc.tile_pool(name="av_ps", bufs=2, space="PSUM"))
        exp_pool = ctx.enter_context(tc.tile_pool(name="exp_pool", bufs=29))
        rc_pool = ctx.enter_context(tc.tile_pool(name="rc_pool", bufs=4))
        nm_pool = ctx.enter_context(tc.tile_pool(name="nm_pool", bufs=4))
        oev_pool = ctx.enter_context(tc.tile_pool(name="oev_pool", bufs=2))

        # ---- resident tiles ----
        bqk_sb = res.tile([P, 16], F32)

        if QK_FP8:
            xdr_sb = res.tile([P, 4, 2, N], FP8)
            for t2 in range(4):
                nc.sync.dma_start(xdr_sb[:, t2], xdr[:, t2])
            wqk_sb = res.tile([P, 16, 4, 2, P], FP8)
        else:
            wqk_sb = res.tile([P, 16, 8, P], FP16)
        for co in range(8):
            nc.sync.dma_start(wqk_sb[:, co], wqk[:, co])
            nc.sync.dma_start(wqk_sb[:, 8 + co], wqk[:, 8 + co])
            if co == 0:
                nc.sync.dma_start(bqk_sb[:], bqk[:])

        # wv/x^T/wp/bp follow the wqk chunks on the sync queue in need-order
        # (V-gen ~20us in, projection ~45us in); out-DMAs use the ACT queue
        wv_sb = res.tile([P, 8, D], FP16)
        xt_sb = res.tile([P, 8, N], FP16)
        bv_rep = res.tile([P, D], BF16)
        nc.sync.dma_start(bv_rep[:], bv[None, :].to_broadcast([P, D]))
        for dc in range(8):
            nc.sync.dma_start(wv_sb[:, dc], wv[:, dc])
        for dc in range(8):
            nc.sync.dma_start(xt_sb[:, dc], xt[:, dc])
        wp_sb = res.tile([P, 8, D], FP16)
        bp_rep = res.tile([P, D], BF16)
        for co in range(8):
            nc.sync.dma_start(wp_sb[:, co], wp[:, co])
        nc.sync.dma_start(bp_rep[:], bp[None, :].to_broadcast([P, D]))

        # v_aug[p, kt, h, :]: cols 0:64 = v for head h at key chunk kt,
        # col 64 = 1.0 (flipped attn@v then emits the softmax denominator
        # in output column 64, one value per q-partition)
        v_aug = res.tile([P, 8, H, HD + 1], FP16)
        nc.gpsimd.memset(v_aug[:, :, :, HD : HD + 1], 1.0)

        qk_sb = res.tile([P, 16, N], FP16)      # co 0..7 = q chunks, 8..15 = k
        # transposed normalized att, packed for proj: [d-part, qc, co, q]
        nmT = res.tile([P, 8, 8, P], FP16)

        # ---------------- emission helpers ----------------
        def qk_nh(co, nh):
            # half of a qk chunk: one psum group + bias evac (GPSIMD cannot
            # read PSUM on real hardware, so evacs run on DVE)
            ps = gen_ps.tile([P, 512], F32, tag="gps")
            if QK_FP8:
                for t2 in range(4):
                    nc.tensor.matmul(
                        ps[:],
                        lhsT=wqk_sb[:, co, t2],
                        rhs=xdr_sb[:, t2, :, nh * 512 : (nh + 1) * 512],
                        start=(t2 == 0),
                        stop=(t2 == 3),
                        perf_mode=DR,
                    )
            else:
                for dc in range(8):
                    nc.tensor.matmul(
                        ps[:],
                        lhsT=wqk_sb[:, co, dc],
                        rhs=xt_sb[:, dc, nh * 512 : (nh + 1) * 512],
                        start=(dc == 0),
                        stop=(dc == 7),
                    )
            nc.vector.tensor_scalar_add(
                qk_sb[:, co, nh * 512 : (nh + 1) * 512],
                ps[:],
                bqk_sb[:, co : co + 1],
            )

        def v_halves(eh, nt):
            # v[keys nt-chunk, 512 cols of eh] split into two PE units
            # sharing one psum accumulation group
            hold = []

            def a():
                ps = gen_ps.tile([P, 512], F32, tag="gps")
                hold.append(ps)
                for dc in range(4):
                    nc.tensor.matmul(
                        ps[:],
                        lhsT=xt_sb[:, dc, nt * P : (nt + 1) * P],
                        rhs=wv_sb[:, dc, eh * 512 : (eh + 1) * 512],
                        start=(dc == 0),
                        stop=False,
                    )

            def b():
                ps = hold[0]
                for dc in range(4, 8):
                    nc.tensor.matmul(
                        ps[:],
                        lhsT=xt_sb[:, dc, nt * P : (nt + 1) * P],
                        rhs=wv_sb[:, dc, eh * 512 : (eh + 1) * 512],
                        start=False,
                        stop=(dc == 7),
                    )
                nc.vector.tensor_add(
                    v_aug[:, nt, eh * 8 : (eh + 1) * 8, 0:HD],
                    ps[:].rearrange("p (h d) -> p h d", d=HD),
                    bv_rep[:, eh * 512 : (eh + 1) * 512].rearrange(
                        "p (h d) -> p h d", d=HD
                    ),
                )

            return a, b

        def sc_group(qh, co, g, hh):
            # one kt-group of scores + its batched exp; returns the exp tile
            q0 = qh * QW
            b0 = hh * HD
            ps = sc_ps.tile([P, 4 * QW], F32, tag="scps")
            for ki in range(4):
                kt = g * 4 + ki
                nc.tensor.matmul(
                    ps[:, ki * QW : (ki + 1) * QW],
                    lhsT=qk_sb[b0 : b0 + HD, 8 + co, kt * P : (kt + 1) * P],
                    rhs=qk_sb[b0 : b0 + HD, co, q0 : q0 + QW],
                    start=True,
                    stop=True,
                )
            ex = exp_pool.tile([P, 4, QW], FP16, tag="exp")
            nc.scalar.activation(
                ex[:], ps[:].rearrange("p (k q) -> p k q", k=4),
                EXP, scale=EXP_SCALE,
            )
            return ex

        def av_halves(qh, co, exps):
            # flipped attn@v for one head pair, split per head; the second
            # half also emits the batched XBAR transpose into nmT
            hold = []

            def half(hh):
                h = 2 * co + hh
                if hh == 0:
                    hold.append(nm_pool.tile([P, 2, 2, HD], FP16, tag="nm", name="nm"))
                nm = hold[0]
                for qs in range(2):
                    ps = av_ps.tile([P, HD + 1], F32, tag="avps")
                    for kt in range(8):
                        nc.tensor.matmul(
                            ps[:],
                            lhsT=exps[(hh, kt // 4)][:, kt % 4,
                                                     qs * P : (qs + 1) * P],
                            rhs=v_aug[:, kt, h, :],
                            start=(kt == 0),
                            stop=(kt == 7),
                        )
                    rc = rc_pool.tile([P, 1], F32, tag="rc", name="rc")
                    nc.vector.reciprocal(rc[:], ps[:, HD : HD + 1])
                    nc.vector.tensor_scalar_mul(
                        nm[:, qs, hh, :], ps[:, 0:HD], rc[:]
                    )
                if hh == 1:
                    nc.sync.dma_start(
                        nmT[:, qh * 2 : qh * 2 + 2, co, :],
                        nm[:].rearrange("p a b d -> p (a b d)"),
                        transpose=True,
                    )

            return (lambda: half(0)), (lambda: half(1))

        def pj_halves(qh, nt, fh):
            # one projection output group split into two PE units
            qc = qh * 2 + nt
            n0 = qc * P
            hold = []

            def a():
                ps = gen_ps.tile([P, 512], F32, tag="gps")
                hold.append(ps)
                for co in range(4):
                    nc.tensor.matmul(
                        ps[:],
                        lhsT=nmT[:, qc, co, :],
                        rhs=wp_sb[:, co, fh * 512 : (fh + 1) * 512],
                        start=(co == 0),
                        stop=False,
                    )

            def b():
                ps = hold[0]
                for co in range(4, 8):
                    nc.tensor.matmul(
                        ps[:],
                        lhsT=nmT[:, qc, co, :],
                        rhs=wp_sb[:, co, fh * 512 : (fh + 1) * 512],
                        start=False,
                        stop=(co == 7),
                    )
                ev = oev_pool.tile([P, 512], F32, tag="oev")
                nc.vector.tensor_add(
                    ev[:], ps[:], bp_rep[:, fh * 512 : (fh + 1) * 512]
                )
                nc.scalar.dma_start(
                    out[n0 : n0 + P, fh * 512 : (fh + 1) * 512], ev[:]
                )

            return a, b

        # ---------------- schedule ----------------
        # Greedy merge with virtual clocks: pe_t/act_t track each engine's
        # busy-until time under the cost model (score group 428ns PE +
        # 1038ns ACT; filler units carry their PE cost). Filler is emitted
        # while ACT has >1.3us of backlog; otherwise the next score group
        # goes out. sc_ps double-buffering caps how far PE runs ahead.
        from collections import deque

        qk_nh(0, 0)
        qk_nh(0, 1)
        qk_nh(8, 0)
        qk_nh(8, 1)

        v_units = [(eh, nt) for eh in range(2) for nt in range(8)]
        vi = 0
        av_q = deque()
        proj_q = deque()
        fq = deque()            # (cost_ns, thunk, exp_tiles_freed)
        pe_t = 4400.0           # first score matmul ~ after xdr+wqk DMA
        act_t = 0.0
        exp_starts = []

        def av_ready(nvi):
            if not av_q:
                return False
            qh0, co0, _ = av_q[0]
            need = 8 if (qh0 == 0 and co0 < 4) else 16
            return nvi >= need

        for qh in range(NQ):
            for co in range(8):
                it = qh * 8 + co
                if qh == 0 and co >= 1:
                    for nh in range(2):
                        fq.append((428, (lambda c, n: lambda: qk_nh(c, n))(co, nh), 0))
                        fq.append(
                            (428, (lambda c, n: lambda: qk_nh(8 + c, n))(co, nh), 0)
                        )
                if pe_t > 56000 and vi < 16:
                    for _ in range(2):
                        if vi < 16:
                            a, b = v_halves(*v_units[vi])
                            fq.append((852, a, 0))
                            fq.append((852, b, 0))
                            vi += 1
                keep = 4 if it < 28 else 1
                n = 0
                while len(av_q) > keep and n < 3 and av_ready(vi):
                    item = av_q.popleft()
                    a, b = av_halves(*item)
                    fq.append((440, a, 0))
                    fq.append((440, b, 4))
                    n += 1
                    if item[1] == 7:
                        for nt in range(2):
                            for fh in range(2):
                                proj_q.append((item[0], nt, fh))
                if proj_q:
                    a, b = pj_halves(*proj_q.popleft())
                    fq.append((852, a, 0))
                    fq.append((852, b, 0))
                # exp-pool liveness guard: queued-but-unexecuted AV halves
                # keep exp tiles alive; force-drain before allocating 4 more
                av_fq = sum(e[2] for e in fq)
                while fq and 4 * len(av_q) + av_fq + 8 > 29:
                    c, t, fr = fq.popleft()
                    t()
                    pe_t += c
                    av_fq -= fr
                exps = {}
                for g in range(2):
                    for hh in range(2):
                        ni = len(exp_starts)
                        cap = exp_starts[ni - 2] + 1038 if ni >= 2 else 0.0
                        while pe_t < cap - 50 or (act_t - pe_t) > 1300:
                            if not fq:
                                if proj_q:
                                    pa, pb = pj_halves(*proj_q.popleft())
                                    fq.append((852, pa, 0))
                                    fq.append((852, pb, 0))
                                else:
                                    break
                            c, t, _ = fq.popleft()
                            t()
                            pe_t += c
                        pe_t = max(pe_t, cap) + 428
                        st = max(act_t, pe_t)
                        exp_starts.append(st)
                        act_t = st + 1038
                        exps[(hh, g)] = sc_group(qh, co, g, hh)
                av_q.append((qh, co, exps))
        while av_q:
            a, b = av_halves(*av_q.popleft())
            a()
            b()
        while fq:
            fq.popleft()[1]()
        proj_q.extend((NQ - 1, nt, fh) for nt in range(2) for fh in range(2))
        while proj_q:
            a, b = pj_halves(*proj_q.popleft())
            a()
            b()





def make_in_maps(x, c, kv_w, kv_b, shared_q_w, shared_q_b, cohort_q_w, cohort_q_b,
                 proj_w, proj_b):
    f32 = np.float32
    fp16 = np.float16
    fp8 = mybir.dt.np(FP8)
    x = np.asarray(x, dtype=f32)
    c = np.asarray(c).astype(np.int64)
    kv_w = np.asarray(kv_w, dtype=f32)
    kv_b = np.asarray(kv_b, dtype=f32)
    shared_q_w = np.asarray(shared_q_w, dtype=f32)
    shared_q_b = np.asarray(shared_q_b, dtype=f32)
    cohort_q_w = np.asarray(cohort_q_w, dtype=f32)
    cohort_q_b = np.asarray(cohort_q_b, dtype=f32)
    proj_w = np.asarray(proj_w, dtype=f32)
    proj_b = np.asarray(proj_b, dtype=f32)

    wk = kv_w[:D] * WS
    wv_ = kv_w[D:]
    bk = kv_b[:D] * WS
    bv_ = kv_b[D:]

    wv_h = np.ascontiguousarray(
        wv_.T.reshape(8, P, D).transpose(1, 0, 2)
    ).astype(fp16)
    wp_h = np.ascontiguousarray(
        proj_w.T.reshape(8, P, D).transpose(1, 0, 2)
    ).astype(fp16)

    in_maps = []
    for b in range(x.shape[0]):
        wq = np.concatenate([shared_q_w, cohort_q_w[c[b]]], axis=0) * WS
        bq = np.concatenate([shared_q_b, cohort_q_b[c[b]]], axis=0) * WS
        wqk_cols = np.concatenate([wq, wk], axis=0)     # [2048 e, 1024 d]
        if QK_FP8:
            # [p, co, t2, j, ec]
            wqk_h = np.ascontiguousarray(
                wqk_cols.T.reshape(4, 2, P, 16, P).transpose(2, 3, 0, 1, 4)
            ).astype(fp8)
        else:
            # [p, co, dc, ec]
            wqk_h = np.ascontiguousarray(
                wqk_cols.T.reshape(8, P, 16, P).transpose(1, 2, 0, 3)
            ).astype(fp16)
        bqk_h = np.ascontiguousarray(
            np.concatenate([bq, bk]).reshape(16, P).T
        ).astype(f32)
        xt_h = np.ascontiguousarray(
            x[b].T.reshape(8, P, N).transpose(1, 0, 2)
        ).astype(fp16)
        m = {
            "xt": xt_h,
            "wqk": wqk_h,
            "bqk": bqk_h,
            "wv": wv_h,
            "bv": np.ascontiguousarray(bv_).astype(mybir.dt.np(BF16)),
            "wp": wp_h,
            "bp": np.ascontiguousarray(proj_b).astype(mybir.dt.np(BF16)),
        }
        if QK_FP8:
            m["xdr"] = np.ascontiguousarray(
                x[b].T.reshape(4, 2, P, N).transpose(2, 0, 1, 3)
            ).astype(fp8)
        in_maps.append(m)
    return in_maps


_NC_CACHE = {}


def kernel(**inputs) -> np.ndarray:
    in_maps = make_in_maps(**inputs)
    if "nc" not in _NC_CACHE:
        _NC_CACHE["nc"] = build_nc()
    nc = _NC_CACHE["nc"]
    res = run_bass_kernel_spmd(nc, in_maps, core_ids=list(range(NCORES)))
    out = np.stack([res.results[i]["out"] for i in range(NCORES)], axis=0)
    return out.astype(np.float32)



# revision 6
# speedup vs baseline: 1.0077x; 1.0077x over previous
"""CohortAwareBlock Trainium2 kernel.

Data-parallel over batch B=8 across 8 NeuronCores (one sample per core).
Cohort routing (gather of cohort_q_w by per-sample cohort id) happens on the
host while building each core's weight tensors; the device kernel is a plain
attention block.

Numerics:
  - QK-gen runs as fp8-e4m3 DoubleRow matmuls (weights pre-scaled x32 to
    dodge fp8 subnormals; the inverse scale is folded into the exp scale).
  - q/k are stored as fp8 in a DoubleRow-interleaved layout ([32, 2, N] per
    head, 4 heads stacked across 128 partitions) so the scores matmul also
    runs fp8-DR: 2x fewer PE cycles than fp16 scores.
  - exp is split across the ACT engine (exact table exp, fp16 out) and the
    DVE (Schraudolph bit-trick: y = int16(A*s + B) bit-read as fp16, ~1.8%
    rms sawtooth error) so neither engine is the bottleneck.
  - v / attn weights / projection stay fp16.

Per-core structure:
  q4k4 [128, 8, 2, N] fp8  (4 q-head groups + 4 k-head groups, DR layout)
  v_aug [keys, h, 65] fp16 (col 64 = 1.0 so the flipped AV emits the
                            softmax denominator per q-partition)
  per (q-quarter, head pair):
    scores -> 2-bank PSUM [128, 4, 256] via fp8-DR -> exp (ACT fp16 or DVE
    Schraudolph, routed by backlog) ->
    flipped attn@v: av_ps [128, 2, 2, 65]; col 64 = den ->
    batched DVE reciprocal [128,2,2,1] + broadcast mult -> nm fp16 ->
    DMA-XBAR transpose -> nmT [d, q] -> proj (fp16) + bias

PE emission is software-pipelined with virtual engine clocks (pe/act/dve);
QK/V generation and the projection fill PE slack under the exp window, and
dummy warmup matmuls keep the PE p-state ramped before the first real work.
"""

import numpy as np

import concourse.bass as bass
import concourse.bacc as bacc
import concourse.mybir as mybir
import concourse.tile as tile
from concourse.bass_utils import run_bass_kernel_spmd

P = 128
N = 1024            # sequence length
D = 1024            # model dim
H = 16              # heads
HD = 64             # head dim
NQ = 4              # q-quarters (256 q each)
QW = N // NQ        # 256
SCALE = HD ** -0.5
NCORES = 8

WS = 32.0           # fp8 pre-scale on w_q/w_k (and so on q/k values)
EXP_SCALE = SCALE / (WS * WS)

# Schraudolph fp16-bitcast exp on DVE: y_bits = int16(s * A + B); bits read
# as fp16 give exp(s*EXP_SCALE) with ~1.8% rms sawtooth error.
LOG2E = 1.4426950408889634
SCHR_A = EXP_SCALE * LOG2E * 1024.0
SCHR_B = 15301.0
SCHR_MAX = 26        # max exp groups routed to DVE (of 128); error budget cap

F32 = mybir.dt.float32
FP16 = mybir.dt.float16
BF16 = mybir.dt.bfloat16
FP8 = mybir.dt.float8e4
I16 = mybir.dt.int16
DR = mybir.MatmulPerfMode.DoubleRow
EXP = mybir.ActivationFunctionType.Exp
MUL = mybir.AluOpType.mult
ADD = mybir.AluOpType.add


def build_nc():
    nc = bacc.Bacc(
        "TRN2",
        target_bir_lowering=False,
        debug=False,
        num_devices=NCORES,
    )

    # ---- external I/O (per-core shards, host-prepped layouts) ----
    # DoubleRow-interleaved d-dim: d = (t2*2 + dj)*128 + p
    xdr = nc.dram_tensor("xdr", [P, 4, 2, N], FP8, kind="ExternalInput")
    # wqk[p, g, j, t2, dj, ec]: g = 4-head group (0..3 q, 4..7 k); j = d-half
    # of the head (e_local = j*32 + i); ec = hh*32 + i -> head 4*(g%4)+hh.
    wqk = nc.dram_tensor("wqk", [P, 8, 2, 4, 2, P], FP8, kind="ExternalInput")
    bqk = nc.dram_tensor("bqk", [P, 8, 2], F32, kind="ExternalInput")
    xt = nc.dram_tensor("xt", [P, 8, N], FP16, kind="ExternalInput")   # x^T
    wv = nc.dram_tensor("wv", [P, 8, D], FP16, kind="ExternalInput")
    bv = nc.dram_tensor("bv", [D], BF16, kind="ExternalInput")
    wp = nc.dram_tensor("wp", [P, 8, D], FP16, kind="ExternalInput")
    bp = nc.dram_tensor("bp", [D], BF16, kind="ExternalInput")
    out = nc.dram_tensor("out", [N, D], F32, kind="ExternalOutput")

    with tile.TileContext(nc) as tc:
        kernel_body(tc, xdr, wqk, bqk, xt, wv, bv, wp, bp, out)
    nc.compile()
    return nc


def kernel_body(tc, xdr, wqk, bqk, xt, wv, bv, wp, bp, out):
    nc = tc.nc
    from contextlib import ExitStack

    with ExitStack() as ctx:
        ctx.enter_context(
            nc.allow_low_precision(reason="fp16/fp8 matmul inputs by design")
        )
        res = ctx.enter_context(tc.tile_pool(name="res", bufs=1))
        gen_ps = ctx.enter_context(tc.tile_pool(name="gen_ps", bufs=2, space="PSUM"))
        sc_ps = ctx.enter_context(tc.tile_pool(name="sc_ps", bufs=2, space="PSUM"))
        av_ps = ctx.enter_context(tc.tile_pool(name="av_ps", bufs=2, space="PSUM"))
        exp_pool = ctx.enter_context(tc.tile_pool(name="exp_pool", bufs=34))
        rc_pool = ctx.enter_context(tc.tile_pool(name="rc_pool", bufs=4))
        nm_pool = ctx.enter_context(tc.tile_pool(name="nm_pool", bufs=4))
        oev_pool = ctx.enter_context(tc.tile_pool(name="oev_pool", bufs=2))

        # ---- resident tiles ----
        warm = res.tile([1, 513], FP16)
        nc.gpsimd.memset(warm[:], 1.0)

        xdr_sb = res.tile([P, 4, 2, N], FP8)
        wqk_sb = res.tile([P, 8, 2, 4, 2, P], FP8)
        bqk_sb = res.tile([P, 8, 2], F32)
        # q/k in scores-DR layout: group g (0..3 q, 4..7 k), partition
        # (hh*32+i), j, token -> value of head 4*(g%4)+hh, d = j*32+i
        q4k4 = res.tile([P, 8, 2, N], FP8)
        xt_sb = res.tile([P, 8, N], FP16)
        wv_sb = res.tile([P, 8, D], FP16)
        bv_rep = res.tile([P, D], BF16)
        wp_sb = res.tile([P, 8, D], FP16)
        bp_rep = res.tile([P, D], BF16)

        # v_aug[p, nt, h, :]: cols 0:64 = v for head h at key chunk nt,
        # col 64 = 1.0 (flipped attn@v then emits the softmax denominator
        # in output column 64, one value per q-partition)
        v_aug = res.tile([P, 8, H, HD + 1], FP16)
        nc.gpsimd.memset(v_aug[:, :, :, HD : HD + 1], 1.0)

        # transposed normalized att, packed for proj: [d-part, qc, co, q]
        nmT = res.tile([P, 8, 8, P], FP16)

        # ---- input DMAs (sync queue, need-order) ----
        nc.sync.dma_start(xdr_sb[:], xdr[:])
        for g in (0, 4):
            for j in range(2):
                nc.sync.dma_start(wqk_sb[:, g, j], wqk[:, g, j])
        nc.sync.dma_start(bqk_sb[:], bqk[:])
        for g in (1, 5):
            for j in range(2):
                nc.sync.dma_start(wqk_sb[:, g, j], wqk[:, g, j])
        for dc in range(8):
            nc.sync.dma_start(wv_sb[:, dc], wv[:, dc])
        nc.sync.dma_start(bv_rep[:], bv[None, :].to_broadcast([P, D]))
        for g in (2, 6):
            for j in range(2):
                nc.sync.dma_start(wqk_sb[:, g, j], wqk[:, g, j])
        for dc in range(8):
            nc.sync.dma_start(xt_sb[:, dc], xt[:, dc])
        for g in (3, 7):
            for j in range(2):
                nc.sync.dma_start(wqk_sb[:, g, j], wqk[:, g, j])
        for co in range(8):
            nc.sync.dma_start(wp_sb[:, co], wp[:, co])
        nc.sync.dma_start(bp_rep[:], bp[None, :].to_broadcast([P, D]))

        # ---------------- emission helpers ----------------
        def warmup():
            # keep the PE p-state ramped while input DMAs land
            ps = gen_ps.tile([P, 512], F32, tag="gps", name="gps")
            nc.tensor.matmul(
                ps[0:1, :],
                lhsT=warm[:, 512:513],
                rhs=warm[:, 0:512],
                start=True,
                stop=True,
            )

        def qk_unit(g, j, ch):
            # one QK-gen psum group: 4 fp8-DR matmuls + biased fp8 evac into
            # the scores-DR layout (GPSIMD cannot read PSUM, so evac on DVE)
            ps = gen_ps.tile([P, 512], F32, tag="gps", name="gps")
            for t2 in range(4):
                nc.tensor.matmul(
                    ps[:],
                    lhsT=wqk_sb[:, g, j, t2],
                    rhs=xdr_sb[:, t2, :, ch * 512 : (ch + 1) * 512],
                    start=(t2 == 0),
                    stop=(t2 == 3),
                    perf_mode=DR,
                )
            nc.vector.tensor_scalar_add(
                q4k4[:, g, j, ch * 512 : (ch + 1) * 512],
                ps[:],
                bqk_sb[:, g, j : j + 1],
            )

        def v_halves(eh, nt):
            # v[keys nt-chunk, 512 cols of eh] split into two PE units
            # sharing one psum accumulation group
            hold = []

            def a():
                ps = gen_ps.tile([P, 512], F32, tag="gps", name="gps")
                hold.append(ps)
                for dc in range(4):
                    nc.tensor.matmul(
                        ps[:],
                        lhsT=xt_sb[:, dc, nt * P : (nt + 1) * P],
                        rhs=wv_sb[:, dc, eh * 512 : (eh + 1) * 512],
                        start=(dc == 0),
                        stop=False,
                    )

            def b():
                ps = hold[0]
                for dc in range(4, 8):
                    nc.tensor.matmul(
                        ps[:],
                        lhsT=xt_sb[:, dc, nt * P : (nt + 1) * P],
                        rhs=wv_sb[:, dc, eh * 512 : (eh + 1) * 512],
                        start=False,
                        stop=(dc == 7),
                    )
                nc.vector.tensor_add(
                    v_aug[:, nt, eh * 8 : (eh + 1) * 8, 0:HD],
                    ps[:].rearrange("p (h d) -> p h d", d=HD),
                    bv_rep[:, eh * 512 : (eh + 1) * 512].rearrange(
                        "p (h d) -> p h d", d=HD
                    ),
                )

            return a, b

        def sc_group(qh, co, g, hh, use_dve):
            # one kt-group of scores (fp8-DR) + its batched exp (ACT exact
            # or DVE Schraudolph); returns the exp tile
            h = 2 * co + hh
            grp = h // 4
            r = 32 * (h % 4)
            q0 = qh * QW
            ps = sc_ps.tile([P, 4, QW], F32, tag="scps", name="scps")
            for ki in range(4):
                kt = g * 4 + ki
                nc.tensor.matmul(
                    ps[:, ki],
                    lhsT=q4k4[r : r + 32, 4 + grp, :, kt * P : (kt + 1) * P],
                    rhs=q4k4[r : r + 32, grp, :, q0 : q0 + QW],
                    start=True,
                    stop=True,
                    perf_mode=DR,
                    tile_position=(r, 0),
                )
            ex = exp_pool.tile([P, 4, QW], FP16, tag="exp", name="exp")
            if use_dve:
                nc.vector.tensor_scalar(
                    ex[:].bitcast(I16),
                    ps[:],
                    SCHR_A,
                    SCHR_B,
                    op0=MUL,
                    op1=ADD,
                )
            else:
                nc.scalar.activation(ex[:], ps[:], EXP, scale=EXP_SCALE)
            return ex

        def av_halves(qh, co, exps):
            # flipped attn@v for one head pair, split per head; the batched
            # norm + XBAR transpose runs after the second half
            hold = []

            def half(hh):
                h = 2 * co + hh
                if hh == 0:
                    hold.append(av_ps.tile([P, 2, 2, HD + 1], F32, tag="avps", name="avps"))
                ps = hold[0]
                for qs in range(2):
                    for kt in range(8):
                        nc.tensor.matmul(
                            ps[:, qs, hh],
                            lhsT=exps[(hh, kt // 4)][:, kt % 4,
                                                     qs * P : (qs + 1) * P],
                            rhs=v_aug[:, kt, h, :],
                            start=(kt == 0),
                            stop=(kt == 7),
                        )
                if hh == 1:
                    rc = rc_pool.tile([P, 2, 2, 1], F32, tag="rc", name="rc")
                    nc.vector.reciprocal(rc[:], ps[:, :, :, HD : HD + 1])
                    nm = nm_pool.tile([P, 2, 2, HD], FP16, tag="nm", name="nm")
                    nc.vector.tensor_tensor(
                        nm[:],
                        ps[:, :, :, 0:HD],
                        rc[:].broadcast_to([P, 2, 2, HD]),
                        op=MUL,
                    )
                    nc.sync.dma_start(
                        nmT[:, qh * 2 : qh * 2 + 2, co, :],
                        nm[:].rearrange("p a b d -> p (a b d)"),
                        transpose=True,
                    )

            return (lambda: half(0)), (lambda: half(1))

        def pj_halves(qh, nt, fh):
            # one projection output group split into two PE units
            qc = qh * 2 + nt
            n0 = qc * P
            hold = []

            def a():
                ps = gen_ps.tile([P, 512], F32, tag="gps", name="gps")
                hold.append(ps)
                for co in range(4):
                    nc.tensor.matmul(
                        ps[:],
                        lhsT=nmT[:, qc, co, :],
                        rhs=wp_sb[:, co, fh * 512 : (fh + 1) * 512],
                        start=(co == 0),
                        stop=False,
                    )

            def b():
                ps = hold[0]
                for co in range(4, 8):
                    nc.tensor.matmul(
                        ps[:],
                        lhsT=nmT[:, qc, co, :],
                        rhs=wp_sb[:, co, fh * 512 : (fh + 1) * 512],
                        start=False,
                        stop=(co == 7),
                    )
                ev = oev_pool.tile([P, 512], F32, tag="oev", name="oev")
                nc.vector.tensor_add(
                    ev[:], ps[:], bp_rep[:, fh * 512 : (fh + 1) * 512]
                )
                nc.sync.dma_start(
                    out[n0 : n0 + P, fh * 512 : (fh + 1) * 512], ev[:]
                )

            return a, b

        # ---------------- schedule ----------------
        # Greedy merge with virtual clocks: pe_t/act_t/dve_t track each
        # engine's busy-until time under the cost model. Filler units (QK/V
        # gen, projection) are emitted while the exp engines have backlog;
        # exp groups are routed ACT vs DVE (Schraudolph) by backlog, capped
        # at SCHR_MAX for the error budget. sc_ps double-buffering caps how
        # far PE runs ahead (cap = finish of the exp two groups back).
        from collections import deque

        C_SC = 220.0          # score group PE (4 fp8-DR matmuls)
        C_EXP_ACT = 1038.0
        C_EXP_DVE = 1237.0
        C_AVH = 440.0         # AV half PE
        C_NORM = 800.0        # batched recip+mult DVE
        C_QK = 430.0          # QK unit PE
        C_QK_EV = 750.0       # QK evac DVE
        C_VH = 852.0          # V half PE
        C_V_EV = 750.0        # V evac DVE (on b half)
        C_PJH = 852.0         # proj half PE
        C_PJ_EV = 705.0       # proj evac DVE (on b half)

        for _ in range(8):
            warmup()

        # QK units for group pair (g0, g4): enough for co 0..1 scores
        qk_first = [(0, 0, 0), (0, 1, 0), (4, 0, 0), (4, 1, 0),
                    (4, 0, 1), (4, 1, 1), (0, 0, 1), (0, 1, 1)]
        # remaining QK units in DMA-arrival order; est = earliest start (ns)
        qk_rest = []
        for grp, est in ((1, 6200.0), (2, 13700.0), (3, 21000.0)):
            for (g, ch) in ((grp, 0), (4 + grp, 0), (4 + grp, 1), (grp, 1)):
                for j in range(2):
                    qk_rest.append((g, j, ch, est))

        pe_t = 3900.0
        act_t = 0.0
        dve_t = 0.0
        for u in qk_first:
            qk_unit(*u)
            pe_t += C_QK
            dve_t = max(dve_t, pe_t + 100.0) + C_QK_EV

        # filler queue: (est, pe_cost, dve_cost, thunk, exp_frees, kind)
        fq = deque()
        for (g, j, ch, est) in qk_rest:
            fq.append(
                (est, C_QK, C_QK_EV,
                 (lambda gg, jj, cc: lambda: qk_unit(gg, jj, cc))(g, j, ch),
                 0, "qk")
            )
        v_units = [(eh, nt) for eh in range(2) for nt in range(8)]
        for (eh, nt) in v_units:
            a, b = v_halves(eh, nt)
            fq.append((21500.0, C_VH, 0.0, a, 0, "v"))
            fq.append((21500.0, C_VH, C_V_EV, b, 0, "v"))

        av_q = deque()
        proj_q = deque()
        exp_fin = []
        schr_n = 0
        v_pops = 0

        def run_filler():
            nonlocal pe_t, dve_t, v_pops
            est, c_pe, c_dve, t, fr, kind = fq.popleft()
            t()
            pe_t = max(pe_t, est) + c_pe
            if c_dve:
                dve_t = max(dve_t, pe_t + 100.0) + c_dve
            if kind == "v":
                v_pops += 1
            return fr

        def do_av(item):
            nonlocal pe_t, dve_t
            qh0, co0, exps0 = item
            a, b = av_halves(qh0, co0, exps0)
            fq.append((0.0, C_AVH, 0.0, a, 2, "av"))
            fq.append((0.0, C_AVH, C_NORM, b, 2, "av"))
            if co0 == 7:
                for nt in range(2):
                    for fh in range(2):
                        proj_q.append((qh0, nt, fh))

        def av_ready():
            if not av_q:
                return False
            qh0, co0, _ = av_q[0]
            need = 16 if (qh0 == 0 and co0 < 4) else 32
            return v_pops >= need

        for qh in range(NQ):
            for co in range(8):
                it = qh * 8 + co
                keep = 4 if it < 28 else 1
                n = 0
                while len(av_q) > keep and n < 3 and av_ready():
                    do_av(av_q.popleft())
                    n += 1
                if proj_q:
                    a, b = pj_halves(*proj_q.popleft())
                    fq.append((0.0, C_PJH, 0.0, a, 0, "pj"))
                    fq.append((0.0, C_PJH, C_PJ_EV, b, 0, "pj"))
                # exp-pool liveness guard: queued-but-unexecuted AV halves
                # keep exp tiles alive; force-drain before allocating 4 more
                av_fq = sum(e[4] for e in fq)
                while fq and 4 * len(av_q) + av_fq + 8 > 34:
                    av_fq -= run_filler()
                exps = {}
                for g in range(2):
                    for hh in range(2):
                        use_dve = (
                            schr_n < SCHR_MAX
                            and it >= 6
                            and act_t - pe_t > 1100.0
                            and act_t - dve_t > 1300.0
                        )
                        ni = len(exp_fin)
                        cap = exp_fin[ni - 2] if ni >= 2 else 0.0
                        busy_t = dve_t if use_dve else act_t
                        while pe_t < cap - 50 or (busy_t - pe_t) > 1300:
                            if not fq:
                                break
                            run_filler()
                            busy_t = dve_t if use_dve else act_t
                        pe_t = max(pe_t, cap) + C_SC
                        ex = sc_group(qh, co, g, hh, use_dve)
                        if use_dve:
                            schr_n += 1
                            st = max(dve_t, pe_t + 100.0)
                            dve_t = st + C_EXP_DVE
                            exp_fin.append(dve_t)
                        else:
                            st = max(act_t, pe_t + 100.0)
                            act_t = st + C_EXP_ACT
                            exp_fin.append(act_t)
                        exps[(hh, g)] = ex
                av_q.append((qh, co, exps))
        while av_q:
            do_av(av_q.popleft())
        while fq:
            fq.popleft()[3]()
        while proj_q:
            a, b = pj_halves(*proj_q.popleft())
            a()
            b()


def make_in_maps(x, c, kv_w, kv_b, shared_q_w, shared_q_b, cohort_q_w, cohort_q_b,
                 proj_w, proj_b):
    f32 = np.float32
    fp16 = np.float16
    fp8 = mybir.dt.np(FP8)
    x = np.asarray(x, dtype=f32)
    c = np.asarray(c).astype(np.int64)
    kv_w = np.asarray(kv_w, dtype=f32)
    kv_b = np.asarray(kv_b, dtype=f32)
    shared_q_w = np.asarray(shared_q_w, dtype=f32)
    shared_q_b = np.asarray(shared_q_b, dtype=f32)
    cohort_q_w = np.asarray(cohort_q_w, dtype=f32)
    cohort_q_b = np.asarray(cohort_q_b, dtype=f32)
    proj_w = np.asarray(proj_w, dtype=f32)
    proj_b = np.asarray(proj_b, dtype=f32)

    wk = kv_w[:D] * WS
    wv_ = kv_w[D:]
    bk = kv_b[:D] * WS
    bv_ = kv_b[D:]

    wv_h = np.ascontiguousarray(
        wv_.T.reshape(8, P, D).transpose(1, 0, 2)
    ).astype(fp16)
    wp_h = np.ascontiguousarray(
        proj_w.T.reshape(8, P, D).transpose(1, 0, 2)
    ).astype(fp16)

    in_maps = []
    for b in range(x.shape[0]):
        wq = np.concatenate([shared_q_w, cohort_q_w[c[b]]], axis=0) * WS
        bq = np.concatenate([shared_q_b, cohort_q_b[c[b]]], axis=0) * WS
        wqk_cols = np.concatenate([wq, wk], axis=0)     # [2048 e, 1024 d]
        # e = qk*1024 + head*64 + j*32 + i with head = 4*g4 + hh;
        # device wants [p, g(qk,g4), j, t2, dj, ec(hh,i)]
        wqk_e = wqk_cols.reshape(2, 4, 4, 2, 32, D)   # [qk, g4, hh, j, i, d]
        wqk_e = wqk_e.transpose(0, 1, 3, 2, 4, 5).reshape(8, 2, P, D)
        wqk_full = wqk_e.reshape(8, 2, P, 4, 2, P)    # [g, j, ec, t2, dj, p]
        wqk_h = np.ascontiguousarray(
            wqk_full.transpose(5, 0, 1, 3, 4, 2)
        ).astype(fp8)
        bqk_e = np.concatenate([bq, bk]).reshape(2, 4, 4, 2, 32)
        bqk_h = np.ascontiguousarray(
            bqk_e.transpose(0, 1, 3, 2, 4).reshape(8, 2, P).transpose(2, 0, 1)
        ).astype(f32)
        xt_h = np.ascontiguousarray(
            x[b].T.reshape(8, P, N).transpose(1, 0, 2)
        ).astype(fp16)
        xdr_h = np.ascontiguousarray(
            x[b].T.reshape(4, 2, P, N).transpose(2, 0, 1, 3)
        ).astype(fp8)
        m = {
            "xdr": xdr_h,
            "wqk": wqk_h,
            "bqk": bqk_h,
            "xt": xt_h,
            "wv": wv_h,
            "bv": np.ascontiguousarray(bv_).astype(mybir.dt.np(BF16)),
            "wp": wp_h,
            "bp": np.ascontiguousarray(proj_b).astype(mybir.dt.np(BF16)),
        }
        in_maps.append(m)
    return in_maps


_NC_CACHE = {}


def kernel(**inputs) -> np.ndarray:
    in_maps = make_in_maps(**inputs)
    if "nc" not in _NC_CACHE:
        _NC_CACHE["nc"] = build_nc()
    nc = _NC_CACHE["nc"]
    res = run_bass_kernel_spmd(nc, in_maps, core_ids=list(range(NCORES)))
    out = np.stack([res.results[i]["out"] for i in range(NCORES)], axis=0)
    return out.astype(np.float32)
